# revision 32
# baseline (speedup 1.0000x reference)
"""Multi-head attention (b=4, n=2048, dim=768, 12 heads) on 8 TRN2 NeuronCores.

Sharding: core c handles batch c//2 and head-group c%2 (6 of 12 heads).  Each
core computes its heads' contribution projected through its slice of Wo and
returns a partial [2048, 768] f32 output; the host sums core pairs and adds
the bias.  No on-device collectives needed.

All TensorE data is bf16 with f32 PSUM accumulation.  fp8(e4m3) was measured
and rejected (quantization error in a random-sign dot product does not
average out: ~4% on Q/K/V); a DVE Schraudolph exp path exists but is
disabled (DVE_EXP_MOD) -- the DVE FIFO stalled the AV matmuls and
re-throttled the PE.

  P1: KT/QT = W^T x^T feature-major with head pairs stacked 64+64 in the
      partition dim; V token-major in 128-wide head blocks whose column 64
      is constant 1.
  P2: scores TRANSPOSED ST[j,i] = K Q^T (K=64), exp on ACT with scale 1/8
      (packs of 3 j-chunks amortize ACT's per-instruction overhead), then
      OP[., i] += V'^T exp(ST) accumulated over j in PSUM; the ones column
      of V' makes row 64 of OP the softmax denominator l[i] for free.
      Units ordered (pair, ib, head, jc) so both heads of a pair finish
      back-to-back; 1/l rows are parked bf16 in LRT tiles at 32-aligned
      partition bases as each (head, ib) completes.
  P3: no PE transposes: OP rows 0:64 are copied into [128,512] pair tiles
      (odd head partition-shifted to rows 64:128), a K=1 all-ones matmul
      broadcasts each 1/l row across 64 partitions into PSUM, one DVE
      multiply yields bf16 feature-major otn pairs; the output projection
      contracts the three 128-feature pair chunks through Wo (K=128),
      interleaved per i-block with the normalize.
"""
import os
import sys
import types
import numpy as np
import ml_dtypes

B, N, DIM = 4, 2048, 768
HEADS, DH = 12, 64
HPC = 6                # heads per core
FPC = HPC * DH         # 384 features per core
NCORES = 8
KC = DIM // 128        # 6 contraction chunks
FT = FPC // 128        # 3 feature tiles per core
NT = N // 128          # 16 key chunks of 128
IBS = 512              # i-block size
IB = N // IBS          # 4 i-blocks
BF16 = ml_dtypes.bfloat16
F8 = ml_dtypes.float8_e4m3fn
WSCALE = 64.0

EXP_A = float(0.125 * 128 / np.log(2.0))
EXP_B = float(16256 - 5.5)
DVE_EXP_MOD = 10**9        # exp packs with g % MOD == MOD-1 go to DVE

_cache = {}
last_exec_time_ns = None


def _install_ntff_hook():
    try:
        import antenv.axon_hooks  # noqa: F401
        return
    except ImportError:
        pass
    from trn_agent_boot.trn_boot import _ntff_profile_via_ctypes
    hook = _ntff_profile_via_ctypes('/opt/axon/libaxon_pjrt.so')
    mod = types.ModuleType('antenv.axon_hooks')
    mod.get_axon_ntff_profile_hook = lambda: hook
    import antenv
    sys.modules['antenv.axon_hooks'] = mod
    antenv.axon_hooks = mod


def _build_nc():
    from contextlib import ExitStack
    from concourse import bacc
    import concourse.mybir as mybir
    from concourse.tile import TileContext

    dt = mybir.dt
    EXP = mybir.ActivationFunctionType.Exp
    ALU = mybir.AluOpType

    nc = bacc.Bacc("TRN2", target_bir_lowering=False, debug=False,
                   num_devices=NCORES)
    xT = nc.dram_tensor("xT", [128, KC, N], dt.bfloat16,
                        kind="ExternalInput").ap()
    wq = nc.dram_tensor("wq", [DIM, FPC], dt.bfloat16, kind="ExternalInput").ap()
    wk = nc.dram_tensor("wk", [DIM, FPC], dt.bfloat16, kind="ExternalInput").ap()
    wv = nc.dram_tensor("wv", [DIM, FPC], dt.bfloat16, kind="ExternalInput").ap()
    wo = nc.dram_tensor("wo", [FPC, DIM], dt.bfloat16, kind="ExternalInput").ap()
    out = nc.dram_tensor("out", [N, DIM], dt.float32, kind="ExternalOutput").ap()

    with TileContext(nc) as tc, ExitStack() as ctx:
        const = ctx.enter_context(tc.tile_pool(name="const", bufs=1))
        onesHI = const.tile([128, 64], dt.bfloat16, tag="ohi", name="ohi")
        nc.vector.memset(onesHI[:], 1.0)

        inp = ctx.enter_context(tc.tile_pool(name="inp", bufs=1))
        x3 = [inp.tile([128, KC, N // 2], dt.bfloat16, tag=f"x3_{hf}",
                       name=f"x3_{hf}") for hf in range(2)]
        wqs = [inp.tile([128, FPC], dt.bfloat16, tag=f"wq{k}", name=f"wq{k}")
               for k in range(KC)]
        wks = [inp.tile([128, FPC], dt.bfloat16, tag=f"wk{k}", name=f"wk{k}")
               for k in range(KC)]
        wvs = [inp.tile([128, FPC], dt.bfloat16, tag=f"wv{k}", name=f"wv{k}")
               for k in range(KC)]
        wos = [inp.tile([128, DIM], dt.bfloat16, tag=f"wo{f}", name=f"wo{f}")
               for f in range(FT)]
        nc.sync.dma_start(out=x3[0][:], in_=xT[:, :, 0:N // 2])
        for k in range(KC):
            nc.scalar.dma_start(out=wvs[k][:], in_=wv[k * 128:(k + 1) * 128, :])
        nc.sync.dma_start(out=x3[1][:], in_=xT[:, :, N // 2:N])
        for k in range(KC):
            nc.sync.dma_start(out=wks[k][:], in_=wk[k * 128:(k + 1) * 128, :])
            nc.sync.dma_start(out=wqs[k][:], in_=wq[k * 128:(k + 1) * 128, :])
        for f in range(FT):
            nc.scalar.dma_start(out=wos[f][:], in_=wo[f * 128:(f + 1) * 128, :])

        kqv = ctx.enter_context(tc.tile_pool(name="kqv", bufs=1))
        KT = [kqv.tile([128, N], dt.bfloat16, tag=f"kt{f}", name=f"kt{f}")
              for f in range(FT)]
        QT = [kqv.tile([128, N], dt.bfloat16, tag=f"qt{f}", name=f"qt{f}")
              for f in range(FT)]
        VP = [kqv.tile([128, HPC * 128], dt.bfloat16, tag=f"vp{t}", name=f"vp{t}")
              for t in range(NT)]
        opsb = ctx.enter_context(tc.tile_pool(name="opsb", bufs=1))
        OPSP = [[opsb.tile([128, IBS], dt.float32, tag=f"op{p}_{ib}",
                           name=f"op{p}_{ib}") for ib in range(IB)]
                for p in range(HPC // 2)]
        otnb = ctx.enter_context(tc.tile_pool(name="otnb", bufs=1))
        OTN = [[otnb.tile([128, IBS], dt.bfloat16, tag=f"ot{p}_{ib}",
                          name=f"ot{p}_{ib}") for ib in range(IB)]
               for p in range(HPC // 2)]
        # 1/l rows parked at 32-aligned partition bases: (h, ib) -> u = h*IB+ib
        # lives in LRT[u//4] row 32*(u%4)
        LRT = [otnb.tile([128, IBS], dt.bfloat16, tag=f"lrt{t}",
                         name=f"lrt{t}") for t in range(HPC * IB // 3)]

        # ---- P1: fp8 DoubleRow projections ----
        for t in range(NT):
            nc.vector.memset(
                VP[t].rearrange("p (h c) -> p h c", c=128)[:, :, 64:65], 1.0)
        with tc.tile_pool(name="p1ps", bufs=4, space="PSUM") as p1:
            for t in range(NT):
                ps = p1.tile([128, FPC], dt.float32, tag="p1", name=f"vps{t}")
                for k in range(KC):
                    nc.tensor.matmul(
                        ps[:],
                        lhsT=x3[t // 8][:, k, (t % 8) * 128:(t % 8 + 1) * 128],
                        rhs=wvs[k][:], start=(k == 0), stop=(k == KC - 1))
                nc.vector.tensor_copy(
                    VP[t].rearrange("p (h c) -> p h c", c=128)[:, :, 0:64],
                    ps.rearrange("p (h c) -> p h c", c=64))
            for W, DST in ((wks, KT), (wqs, QT)):
                for f in range(FT):
                    for q in range(N // 512):
                        ps = p1.tile([128, 512], dt.float32, tag="p1",
                                     name=f"kqps{f}_{q}")
                        for k in range(KC):
                            nc.tensor.matmul(
                                ps[:], lhsT=W[k][:, f * 128:(f + 1) * 128],
                                rhs=x3[q // 2][:, k, (q % 2) * 512:
                                               (q % 2 + 1) * 512],
                                start=(k == 0), stop=(k == KC - 1))
                        nc.vector.tensor_copy(DST[f][:, q * 512:(q + 1) * 512],
                                              ps[:])

        # ---- P2: attention (units ordered pair-major) + fused normalize ----
        PACK = 3
        units = [(2 * p + hh, ib, jc) for p in range(HPC // 2)
                 for ib in range(IB) for hh in range(2) for jc in range(NT)]
        assert len(units) % PACK == 0
        with tc.tile_pool(name="p2st", bufs=2, space="PSUM") as p2st, \
                tc.tile_pool(name="p2op", bufs=2, space="PSUM") as p2op, \
                tc.tile_pool(name="expp", bufs=8) as expp:
            ops = {}
            av_cnt = {}
            av_queue = []

            def emit_avs(entries, ex):
                for u, (h, ib, jc) in entries:
                    cnt = av_cnt.get((h, ib), 0)
                    if cnt == 0:
                        ops[(h, ib)] = p2op.tile([128, IBS], dt.float32,
                                                 tag="op", name=f"opp{h}_{ib}")
                    nc.tensor.matmul(
                        ops[(h, ib)][:],
                        lhsT=VP[jc][:, h * 128:(h + 1) * 128],
                        rhs=ex[:, u * IBS:(u + 1) * IBS],
                        start=(cnt == 0), stop=(cnt == NT - 1))
                    av_cnt[(h, ib)] = cnt + 1
                    if cnt == NT - 1:
                        op = ops.pop((h, ib))
                        p, hh = divmod(h, 2)
                        # 1/l of row 64 parked in LRT at a 32-aligned base
                        u_ = h * IB + ib
                        lb = 32 * (u_ % 3)
                        with nc.allow_low_precision(
                                reason="1/l rounded to bf16 for the "
                                       "broadcast matmul rhs"):
                            nc.vector.reciprocal(
                                LRT[u_ // 3][lb:lb + 1, :], op[64:65, :])
                        # pack rows 0:64 into the pair tile (odd head shifted)
                        nc.vector.tensor_copy(
                            OPSP[p][ib][hh * 64:(hh + 1) * 64, :], op[0:64, :])

            for g in range(len(units) // PACK):
                while av_queue and av_queue[0][0] <= g:
                    _, ex_, entries_ = av_queue.pop(0)
                    emit_avs(entries_, ex_)
                pack = units[g * PACK:(g + 1) * PACK]
                st = p2st.tile([128, PACK * IBS], dt.float32, tag="st",
                               name=f"st{g}")
                for u, (h, ib, jc) in enumerate(pack):
                    ktf, qtf, r0 = KT[h // 2], QT[h // 2], (h % 2) * 64
                    nc.tensor.matmul(
                        st[:, u * IBS:(u + 1) * IBS],
                        lhsT=ktf[r0:r0 + 64, jc * 128:(jc + 1) * 128],
                        rhs=qtf[r0:r0 + 64, ib * IBS:(ib + 1) * IBS],
                        start=True, stop=True)
                ex = expp.tile([128, PACK * IBS], dt.bfloat16, tag="ex",
                               name=f"ex{g}")
                if g % DVE_EXP_MOD == DVE_EXP_MOD - 1:
                    # DVE Schraudolph exp; defer its AV consumers two packs
                    # so the PE never waits on the DVE queue
                    nc.vector.tensor_scalar(ex[:].bitcast(dt.uint16), st[:],
                                            EXP_A, EXP_B, ALU.mult, ALU.add)
                    av_queue.append((g + 2, ex, list(enumerate(pack))))
                else:
                    nc.scalar.activation(ex[:], st[:], EXP, scale=0.125)
                    emit_avs(enumerate(pack), ex)
            while av_queue:
                _, ex_, entries_ = av_queue.pop(0)
                emit_avs(entries_, ex_)

        # ---- P3: 1/l broadcast + normalize, then output projection ----
        with tc.tile_pool(name="p3lr", bufs=2, space="PSUM") as p3lr, \
                tc.tile_pool(name="p3pp", bufs=6, space="PSUM") as p3pp, \
                tc.tile_pool(name="outst", bufs=4) as outst:
            for ib in range(IB):
                for p in range(HPC // 2):
                    lrep = p3lr.tile([128, IBS], dt.float32, tag="lr",
                                     name=f"lrep{p}_{ib}")
                    for hh in range(2):
                        u_ = (2 * p + hh) * IB + ib
                        lb = 32 * (u_ % 3)
                        nc.tensor.matmul(
                            lrep[hh * 64:(hh + 1) * 64, :],
                            lhsT=onesHI[lb:lb + 1, :],
                            rhs=LRT[u_ // 3][lb:lb + 1, :],
                            start=True, stop=True)
                    nc.vector.tensor_tensor(OTN[p][ib][:], OPSP[p][ib][:],
                                            lrep[:], ALU.mult)
                for isub in range(ib * 4, ib * 4 + 4):
                    col = (isub % 4) * 128
                    ob = outst.tile([128, DIM], dt.float32, tag="ob",
                                    name=f"ob{isub}")
                    for half in range(2):
                        pp = p3pp.tile([128, DIM // 2], dt.float32, tag="pp",
                                       name=f"pp{isub}_{half}")
                        for p in range(FT):
                            nc.tensor.matmul(
                                pp[:], lhsT=OTN[p][ib][:, col:col + 128],
                                rhs=wos[p][:, half * 384:(half + 1) * 384],
                                start=(p == 0), stop=(p == FT - 1))
                        nc.scalar.copy(ob[:, half * 384:(half + 1) * 384],
                                       pp[:])
                    nc.sync.dma_start(
                        out=out[isub * 128:(isub + 1) * 128, :], in_=ob[:])

    nc.finalize()
    return nc


def _get_nc():
    if "nc" not in _cache:
        _cache["nc"] = _build_nc()
    return _cache["nc"]


def kernel(x, Wq, Wk, Wv, Wo, bo):
    global last_exec_time_ns
    x = np.asarray(x, dtype=np.float32)
    Wq = np.asarray(Wq, dtype=np.float32)
    Wk = np.asarray(Wk, dtype=np.float32)
    Wv = np.asarray(Wv, dtype=np.float32)
    Wo = np.asarray(Wo, dtype=np.float32)
    bo = np.asarray(bo, dtype=np.float32)

    trace = bool(os.environ.get("BASS_KERNEL_TRACE"))
    if trace:
        _install_ntff_hook()
        import concourse.bass_utils as bass_utils
        bass_utils.upload_artifacts = lambda tmpdir: tmpdir

    nc = _get_nc()

    in_maps = []
    for c in range(NCORES):
        bi, hg = divmod(c, 2)
        s = slice(hg * FPC, (hg + 1) * FPC)
        in_maps.append({
            "xT": np.ascontiguousarray(
                x[bi].T.reshape(KC, 128, N).transpose(1, 0, 2)).astype(BF16),
            "wq": np.ascontiguousarray(Wq[:, s]).astype(BF16),
            "wk": np.ascontiguousarray(Wk[:, s]).astype(BF16),
            "wv": np.ascontiguousarray(Wv[:, s]).astype(BF16),
            "wo": np.ascontiguousarray(Wo[s, :]).astype(BF16),
        })

    from concourse.bass_utils import run_bass_kernel_spmd
    res = run_bass_kernel_spmd(nc, in_maps, list(range(NCORES)), trace=trace)
    last_exec_time_ns = res.exec_time_ns

    parts = [res.results[c]["out"] for c in range(NCORES)]
    full = np.empty((B, N, DIM), np.float32)
    for bi in range(B):
        full[bi] = parts[2 * bi] + parts[2 * bi + 1] + bo[None, :]
    return full


# revision 33
# speedup vs baseline: 1.1771x; 1.1771x over previous
"""Multi-head attention (b=4, n=2048, dim=768, 12 heads) on 8 TRN2 NeuronCores.

Sharding: core c handles batch c//2 and head-group c%2 (6 of 12 heads).  Each
core computes its heads' contribution projected through its slice of Wo and
returns a partial [2048, 768] f32 output; the host sums core pairs and adds
the bias.  No on-device collectives needed.

All TensorE data is bf16 with f32 PSUM accumulation.  fp8(e4m3) was measured
and rejected (quantization error in a random-sign dot product does not
average out: ~4% on Q/K/V); a DVE Schraudolph exp path exists but is
disabled (DVE_EXP_MOD) -- the DVE FIFO stalled the AV matmuls and
re-throttled the PE.

  P1: KT/QT = W^T x^T feature-major with head pairs stacked 64+64 in the
      partition dim; V token-major in 128-wide head blocks whose column 64
      is constant 1.
  P2: scores TRANSPOSED ST[j,i] = K Q^T (K=64), exp on ACT with scale 1/8
      (packs of 3 j-chunks amortize ACT's per-instruction overhead), then
      OP[., i] += V'^T exp(ST) accumulated over j in PSUM; the ones column
      of V' makes row 64 of OP the softmax denominator l[i] for free.
      Units ordered (pair, ib, head, jc) so both heads of a pair finish
      back-to-back; 1/l rows are parked bf16 in LRT tiles at 32-aligned
      partition bases as each (head, ib) completes.
  P3: no PE transposes: OP rows 0:64 are copied into [128,512] pair tiles
      (odd head partition-shifted to rows 64:128), a K=1 all-ones matmul
      broadcasts each 1/l row across 64 partitions into PSUM, one DVE
      multiply yields bf16 feature-major otn pairs; the output projection
      contracts the three 128-feature pair chunks through Wo (K=128),
      interleaved per i-block with the normalize.
"""
import os
import sys
import types
import numpy as np
import ml_dtypes

B, N, DIM = 4, 2048, 768
HEADS, DH = 12, 64
HPC = 6                # heads per core
FPC = HPC * DH         # 384 features per core
NCORES = 8
KC = DIM // 128        # 6 contraction chunks
FT = FPC // 128        # 3 feature tiles per core
NT = N // 128          # 16 key chunks of 128
IBS = 512              # i-block size
IB = N // IBS          # 4 i-blocks
BF16 = ml_dtypes.bfloat16
F8 = ml_dtypes.float8_e4m3fn
WSCALE = 64.0

EXP_A = float(0.125 * 128 / np.log(2.0))
EXP_B = float(16256 - 5.5)
DVE_EXP_MOD = 10**9        # exp packs with g % MOD == MOD-1 go to DVE

_cache = {}
last_exec_time_ns = None


def _install_ntff_hook():
    try:
        import antenv.axon_hooks  # noqa: F401
        return
    except ImportError:
        pass
    from trn_agent_boot.trn_boot import _ntff_profile_via_ctypes
    hook = _ntff_profile_via_ctypes('/opt/axon/libaxon_pjrt.so')
    mod = types.ModuleType('antenv.axon_hooks')
    mod.get_axon_ntff_profile_hook = lambda: hook
    import antenv
    sys.modules['antenv.axon_hooks'] = mod
    antenv.axon_hooks = mod


def _build_nc():
    from contextlib import ExitStack
    from concourse import bacc
    import concourse.mybir as mybir
    from concourse.tile import TileContext

    dt = mybir.dt
    EXP = mybir.ActivationFunctionType.Exp
    ALU = mybir.AluOpType

    nc = bacc.Bacc("TRN2", target_bir_lowering=False, debug=False,
                   num_devices=NCORES)
    xT = nc.dram_tensor("xT", [128, KC, N], dt.bfloat16,
                        kind="ExternalInput").ap()
    wq = nc.dram_tensor("wq", [128, KC, FPC], dt.bfloat16,
                        kind="ExternalInput").ap()
    wk = nc.dram_tensor("wk", [128, KC, FPC], dt.bfloat16,
                        kind="ExternalInput").ap()
    wv = nc.dram_tensor("wv", [128, KC, FPC], dt.bfloat16,
                        kind="ExternalInput").ap()
    wo = nc.dram_tensor("wo", [FPC, DIM], dt.bfloat16, kind="ExternalInput").ap()
    out = nc.dram_tensor("out", [N, DIM], dt.float32, kind="ExternalOutput").ap()

    with TileContext(nc) as tc, ExitStack() as ctx:
        const = ctx.enter_context(tc.tile_pool(name="const", bufs=1))
        onesHI = const.tile([128, 64], dt.bfloat16, tag="ohi", name="ohi")
        nc.vector.memset(onesHI[:], 1.0)

        inp = ctx.enter_context(tc.tile_pool(name="inp", bufs=1))
        x3 = [inp.tile([128, KC, N // 2], dt.bfloat16, tag=f"x3_{hf}",
                       name=f"x3_{hf}") for hf in range(2)]
        wq3 = inp.tile([128, KC, FPC], dt.bfloat16, tag="wq3", name="wq3")
        wk3 = inp.tile([128, KC, FPC], dt.bfloat16, tag="wk3", name="wk3")
        wv3 = inp.tile([128, KC, FPC], dt.bfloat16, tag="wv3", name="wv3")
        wos = [inp.tile([128, DIM], dt.bfloat16, tag=f"wo{f}", name=f"wo{f}")
               for f in range(FT)]
        nc.sync.dma_start(out=x3[0][:], in_=xT[:, :, 0:N // 2])
        nc.scalar.dma_start(out=wv3[:], in_=wv[:, :, :])
        nc.sync.dma_start(out=x3[1][:], in_=xT[:, :, N // 2:N])
        nc.sync.dma_start(out=wk3[:], in_=wk[:, :, :])
        nc.sync.dma_start(out=wq3[:], in_=wq[:, :, :])
        for f in range(FT):
            nc.scalar.dma_start(out=wos[f][:], in_=wo[f * 128:(f + 1) * 128, :])

        kqv = ctx.enter_context(tc.tile_pool(name="kqv", bufs=1))
        KT = [kqv.tile([128, N], dt.bfloat16, tag=f"kt{f}", name=f"kt{f}")
              for f in range(FT)]
        QT = [kqv.tile([128, N], dt.bfloat16, tag=f"qt{f}", name=f"qt{f}")
              for f in range(FT)]
        VP = [kqv.tile([128, HPC * 128], dt.bfloat16, tag=f"vp{t}", name=f"vp{t}")
              for t in range(NT)]
        opsb = ctx.enter_context(tc.tile_pool(name="opsb", bufs=1))
        OPSP = [[opsb.tile([128, IBS], dt.float32, tag=f"op{p}_{ib}",
                           name=f"op{p}_{ib}") for ib in range(IB)]
                for p in range(HPC // 2)]
        otnb = ctx.enter_context(tc.tile_pool(name="otnb", bufs=1))
        OTN = [[otnb.tile([128, IBS], dt.bfloat16, tag=f"ot{p}_{ib}",
                          name=f"ot{p}_{ib}") for ib in range(IB)]
               for p in range(HPC // 2)]
        # 1/l rows parked at 32-aligned partition bases: (h, ib) -> u = h*IB+ib
        # lives in LRT[u//4] row 32*(u%4)
        LRT = [otnb.tile([128, IBS], dt.bfloat16, tag=f"lrt{t}",
                         name=f"lrt{t}") for t in range(HPC * IB // 3)]

        # ---- P1: fp8 DoubleRow projections ----
        for t in range(NT):
            nc.vector.memset(
                VP[t].rearrange("p (h c) -> p h c", c=128)[:, :, 64:65], 1.0)
        with tc.tile_pool(name="p1ps", bufs=4, space="PSUM") as p1:
            for t in range(NT):
                ps = p1.tile([128, FPC], dt.float32, tag="p1", name=f"vps{t}")
                for k in range(KC):
                    nc.tensor.matmul(
                        ps[:],
                        lhsT=x3[t // 8][:, k, (t % 8) * 128:(t % 8 + 1) * 128],
                        rhs=wv3[:, k, :], start=(k == 0), stop=(k == KC - 1))
                nc.vector.tensor_copy(
                    VP[t].rearrange("p (h c) -> p h c", c=128)[:, :, 0:64],
                    ps.rearrange("p (h c) -> p h c", c=64))
            for W, DST in ((wk3, KT), (wq3, QT)):
                for f in range(FT):
                    for q in range(N // 512):
                        ps = p1.tile([128, 512], dt.float32, tag="p1",
                                     name=f"kqps{f}_{q}")
                        for k in range(KC):
                            nc.tensor.matmul(
                                ps[:], lhsT=W[:, k, f * 128:(f + 1) * 128],
                                rhs=x3[q // 2][:, k, (q % 2) * 512:
                                               (q % 2 + 1) * 512],
                                start=(k == 0), stop=(k == KC - 1))
                        nc.vector.tensor_copy(DST[f][:, q * 512:(q + 1) * 512],
                                              ps[:])

        # ---- P2: attention (units ordered pair-major) + fused normalize ----
        PACK = 3
        units = [(2 * p + hh, ib, jc) for p in range(HPC // 2)
                 for ib in range(IB) for hh in range(2) for jc in range(NT)]
        assert len(units) % PACK == 0
        with tc.tile_pool(name="p2st", bufs=2, space="PSUM") as p2st, \
                tc.tile_pool(name="p2op", bufs=2, space="PSUM") as p2op, \
                tc.tile_pool(name="expp", bufs=8) as expp:
            ops = {}
            av_cnt = {}
            av_queue = []

            def emit_avs(entries, ex):
                for u, (h, ib, jc) in entries:
                    cnt = av_cnt.get((h, ib), 0)
                    if cnt == 0:
                        ops[(h, ib)] = p2op.tile([128, IBS], dt.float32,
                                                 tag="op", name=f"opp{h}_{ib}")
                    nc.tensor.matmul(
                        ops[(h, ib)][:],
                        lhsT=VP[jc][:, h * 128:(h + 1) * 128],
                        rhs=ex[:, u * IBS:(u + 1) * IBS],
                        start=(cnt == 0), stop=(cnt == NT - 1))
                    av_cnt[(h, ib)] = cnt + 1
                    if cnt == NT - 1:
                        op = ops.pop((h, ib))
                        p, hh = divmod(h, 2)
                        # 1/l of row 64 parked in LRT at a 32-aligned base
                        u_ = h * IB + ib
                        lb = 32 * (u_ % 3)
                        with nc.allow_low_precision(
                                reason="1/l rounded to bf16 for the "
                                       "broadcast matmul rhs"):
                            nc.vector.reciprocal(
                                LRT[u_ // 3][lb:lb + 1, :], op[64:65, :])
                        # pack rows 0:64 into the pair tile (odd head shifted)
                        nc.vector.tensor_copy(
                            OPSP[p][ib][hh * 64:(hh + 1) * 64, :], op[0:64, :])

            for g in range(len(units) // PACK):
                while av_queue and av_queue[0][0] <= g:
                    _, ex_, entries_ = av_queue.pop(0)
                    emit_avs(entries_, ex_)
                pack = units[g * PACK:(g + 1) * PACK]
                st = p2st.tile([128, PACK * IBS], dt.float32, tag="st",
                               name=f"st{g}")
                for u, (h, ib, jc) in enumerate(pack):
                    ktf, qtf, r0 = KT[h // 2], QT[h // 2], (h % 2) * 64
                    nc.tensor.matmul(
                        st[:, u * IBS:(u + 1) * IBS],
                        lhsT=ktf[r0:r0 + 64, jc * 128:(jc + 1) * 128],
                        rhs=qtf[r0:r0 + 64, ib * IBS:(ib + 1) * IBS],
                        start=True, stop=True)
                ex = expp.tile([128, PACK * IBS], dt.bfloat16, tag="ex",
                               name=f"ex{g}")
                if g % DVE_EXP_MOD == DVE_EXP_MOD - 1:
                    # DVE Schraudolph exp; defer its AV consumers two packs
                    # so the PE never waits on the DVE queue
                    nc.vector.tensor_scalar(ex[:].bitcast(dt.uint16), st[:],
                                            EXP_A, EXP_B, ALU.mult, ALU.add)
                    av_queue.append((g + 2, ex, list(enumerate(pack))))
                else:
                    nc.scalar.activation(ex[:], st[:], EXP, scale=0.125)
                    emit_avs(enumerate(pack), ex)
            while av_queue:
                _, ex_, entries_ = av_queue.pop(0)
                emit_avs(entries_, ex_)

        # ---- P3: 1/l broadcast + normalize, then output projection ----
        with tc.tile_pool(name="p3lr", bufs=2, space="PSUM") as p3lr, \
                tc.tile_pool(name="p3pp", bufs=6, space="PSUM") as p3pp, \
                tc.tile_pool(name="outst", bufs=4) as outst:
            for ib in range(IB):
                for p in range(HPC // 2):
                    lrep = p3lr.tile([128, IBS], dt.float32, tag="lr",
                                     name=f"lrep{p}_{ib}")
                    for hh in range(2):
                        u_ = (2 * p + hh) * IB + ib
                        lb = 32 * (u_ % 3)
                        nc.tensor.matmul(
                            lrep[hh * 64:(hh + 1) * 64, :],
                            lhsT=onesHI[lb:lb + 1, :],
                            rhs=LRT[u_ // 3][lb:lb + 1, :],
                            start=True, stop=True)
                    nc.vector.tensor_tensor(OTN[p][ib][:], OPSP[p][ib][:],
                                            lrep[:], ALU.mult)
                for isub in range(ib * 4, ib * 4 + 4):
                    col = (isub % 4) * 128
                    ob = outst.tile([128, DIM], dt.float32, tag="ob",
                                    name=f"ob{isub}")
                    for half in range(2):
                        pp = p3pp.tile([128, DIM // 2], dt.float32, tag="pp",
                                       name=f"pp{isub}_{half}")
                        for p in range(FT):
                            nc.tensor.matmul(
                                pp[:], lhsT=OTN[p][ib][:, col:col + 128],
                                rhs=wos[p][:, half * 384:(half + 1) * 384],
                                start=(p == 0), stop=(p == FT - 1))
                        nc.scalar.copy(ob[:, half * 384:(half + 1) * 384],
                                       pp[:])
                    nc.sync.dma_start(
                        out=out[isub * 128:(isub + 1) * 128, :], in_=ob[:])

    nc.finalize()
    return nc


def _get_nc():
    if "nc" not in _cache:
        _cache["nc"] = _build_nc()
    return _cache["nc"]


def kernel(x, Wq, Wk, Wv, Wo, bo):
    global last_exec_time_ns
    x = np.asarray(x, dtype=np.float32)
    Wq = np.asarray(Wq, dtype=np.float32)
    Wk = np.asarray(Wk, dtype=np.float32)
    Wv = np.asarray(Wv, dtype=np.float32)
    Wo = np.asarray(Wo, dtype=np.float32)
    bo = np.asarray(bo, dtype=np.float32)

    trace = bool(os.environ.get("BASS_KERNEL_TRACE"))
    if trace:
        _install_ntff_hook()
        import concourse.bass_utils as bass_utils
        bass_utils.upload_artifacts = lambda tmpdir: tmpdir

    nc = _get_nc()

    in_maps = []
    for c in range(NCORES):
        bi, hg = divmod(c, 2)
        s = slice(hg * FPC, (hg + 1) * FPC)
        in_maps.append({
            "xT": np.ascontiguousarray(
                x[bi].T.reshape(KC, 128, N).transpose(1, 0, 2)).astype(BF16),
            "wq": np.ascontiguousarray(
                Wq[:, s].reshape(KC, 128, FPC).transpose(1, 0, 2)).astype(BF16),
            "wk": np.ascontiguousarray(
                Wk[:, s].reshape(KC, 128, FPC).transpose(1, 0, 2)).astype(BF16),
            "wv": np.ascontiguousarray(
                Wv[:, s].reshape(KC, 128, FPC).transpose(1, 0, 2)).astype(BF16),
            "wo": np.ascontiguousarray(Wo[s, :]).astype(BF16),
        })

    from concourse.bass_utils import run_bass_kernel_spmd
    res = run_bass_kernel_spmd(nc, in_maps, list(range(NCORES)), trace=trace)
    last_exec_time_ns = res.exec_time_ns

    parts = [res.results[c]["out"] for c in range(NCORES)]
    full = np.empty((B, N, DIM), np.float32)
    for bi in range(B):
        full[bi] = parts[2 * bi] + parts[2 * bi + 1] + bo[None, :]
    return full
